# revision 20
# baseline (speedup 1.0000x reference)
"""Trainium2 Bass kernel for the BinaryMechanismSSM problem.

Full inputs in, full outputs out. Batch (128) sharded 8 ways (16/core).

Per core:
  Phase 1: projections bx0/bx1/gx = x @ {B0,B1,G}^T + bias as fp16 matmuls
           over 512-token tiles. bx planes are staged to DRAM pre-scaled by
           SCALE=512 (bias folded in); the gate plane is sigmoid(gx) fp16.
  Phase 2: T sequential steps. State held as st[p, 16j+b] = s[b, 128j+p]
           (fp16 [128, 64] tile). A0/A1 are fp8e4 (scaled by SCALE) with
           fp16 rhs; per step 32 A-matmuls + 4 identity-injection matmuls
           accumulate into 4 PSUM quarter tiles (one per state chunk j).
           MM issue order (iden, k=0 blocks, k=1 blocks, then per-j k=2/3
           blocks) lets the per-chunk tanh (scale=1/SCALE) + gate blend
           (DVE, fp16) pipeline underneath the matmuls of later chunks and
           of the next step, keeping the PE continuously busy (HAM-warm).
           States staged out fp16 every 4 steps; host re-layouts to
           [B, T+1, S] fp32.
"""
import numpy as np

B_FULL = 128
T_FULL = 1024
I_DIM = 256
S_DIM = 512
N_CORES = 8
B_LOC = B_FULL // N_CORES  # 16
SCALE = 512.0

_cache = {}


def _build(alpha: float, z: int, T: int):
    import ml_dtypes  # noqa: F401  (ensures fp8 numpy dtypes exist)
    import concourse.bass as bass  # noqa: F401
    from concourse import bacc
    import concourse.mybir as mybir
    from concourse.tile import TileContext

    dt = mybir.dt
    AF = mybir.ActivationFunctionType
    ALU = mybir.AluOpType

    TOK = T * B_LOC          # tokens per core
    NG = T // 16             # step groups
    NMAT = 3 if z != 0 else 2          # number of projection matrices
    NREC = 2 if z != 0 else 1          # number of recurrence matrices
    W2 = NREC * 16           # psum quarter width (m, b)

    nc = bacc.Bacc("TRN2", target_bir_lowering=False, debug=False,
                   num_devices=N_CORES)

    xT_d = nc.declare_dram_parameter("xT", [2, 128, TOK], dt.float16, isOutput=False)
    pw_d = nc.declare_dram_parameter("pw", [128, NMAT * 2 * 4 * 128], dt.float16, isOutput=False)
    bias_d = nc.declare_dram_parameter("bias", [128, 4 * NMAT], dt.float32, isOutput=False)
    aw_d = nc.declare_dram_parameter("aw", [128, NREC * 16 * 128], dt.float8e4, isOutput=False)
    s0_d = nc.declare_dram_parameter("s0T", [128, 64], dt.float16, isOutput=False)
    iden_d = nc.declare_dram_parameter("iden", [128, 128], dt.float8e4, isOutput=False)
    stg_d = nc.declare_dram_parameter("stg", [128, T, 64], dt.float16, isOutput=True)

    with TileContext(nc) as tc:
        # ---------------- fused recurrence + JIT projections ----------
        with (
            tc.tile_pool(name="p2w", bufs=1) as p2w,
            tc.tile_pool(name="p2in", bufs=1) as p2in,
            tc.tile_pool(name="p2st", bufs=1) as p2st,
            tc.tile_pool(name="p2c", bufs=1) as p2c,
            tc.tile_pool(name="p2ps", bufs=1, space="PSUM") as p2ps,
        ):
            aw = p2w.tile([128, NREC * 16 * 128], dt.float8e4)
            nc.sync.dma_start(aw[:], aw_d[:])
            iden = p2w.tile([128, 128], dt.float8e4)
            nc.sync.dma_start(iden[:], iden_d[:])
            s0t = p2w.tile([128, 64], dt.float16)
            nc.sync.dma_start(s0t[:], s0_d[:])
            pw = p2w.tile([128, NMAT * 2 * 4 * 128], dt.float16)
            nc.sync.dma_start(pw[:], pw_d[:])
            bias = p2w.tile([128, 4 * NMAT], dt.float32)
            nc.sync.dma_start(bias[:], bias_d[:])

            a0 = float(1.0 - alpha) if z != 0 else 1.0
            a1 = float(alpha)
            WH = NREC * 32            # psum half width: (m, j2, b)
            ME = NREC + 1             # extended m-dim: mats + carry slice

            # explicit rings (pool buf rotation serializes one generation
            # too tight on the hot path; rings decouple by construction)
            pss_r = [[p2ps.tile([128, WH], dt.float32, tag=f"ps{h}{r}",
                                name=f"ps{h}{r}") for r in range(3)]
                     for h in range(2)]
            pps_r = [p2ps.tile([128, 256], dt.float32, tag=f"pp{r}",
                               name=f"pp{r}") for r in range(2)]
            ft_r = [p2c.tile([128, 2 * WH], dt.float16, tag=f"ft{r}",
                             name=f"ft{r}") for r in range(2)]
            u_r = [[p2c.tile([128, ME * 32], dt.float16, tag=f"u{h}{r}",
                             name=f"u{h}{r}") for r in range(2)]
                   for h in range(2)]
            ob_r = [p2st.tile([128, 4 * 64], dt.float16, tag=f"ob{r}",
                              name=f"ob{r}") for r in range(2)]
            xg_r = [p2in.tile([128, 512], dt.float16, tag=f"xg{r}",
                              name=f"xg{r}") for r in range(2)]
            pjb_r_ = [p2in.tile([128, NREC * 4 * 256], dt.float16,
                                tag=f"pjb{r}", name=f"pjb{r}")
                      for r in range(2)]
            pjg_r_ = [p2in.tile([128, 1024], dt.float16, tag=f"pjg{r}",
                                name=f"pjg{r}") for r in range(2)]
            gco_r_ = [p2in.tile([128, NREC * 1024], dt.float16,
                                tag=f"gco{r}", name=f"gco{r}")
                      for r in range(2)]
            g1m_r_ = [p2in.tile([128, 1024], dt.float16, tag=f"g1m{r}",
                                name=f"g1m{r}") for r in range(2)]

            def stage_xdma(g):
                """Prefetch this group's 256 tokens of x into SBUF."""
                r = g % 2
                for i in range(2):
                    nc.sync.dma_start(xg_r[r][:, i * 256:(i + 1) * 256],
                                      xT_d[i, :, g * 256:(g + 1) * 256])

            # proj chunk order: gate first so gco/g1m can build early
            chunks = [(NMAT - 1, j) for j in range(4)] + \
                     [(m, j) for m in range(NREC) for j in range(4)]

            def stage_proj(g, c):
                """JIT projection chunk c (one (mat, j) pair) for group g."""
                r = g % 2
                mat, j = chunks[c]
                ps = pps_r[c % 2]
                for i in range(2):
                    blk = ((mat * 2 + i) * 4 + j) * 128
                    nc.tensor.matmul(ps[:], pw[:, blk:blk + 128],
                                     xg_r[r][:, i * 256:(i + 1) * 256],
                                     start=(i == 0), stop=(i == 1))
                bj = bias[:, mat * 4 + j:mat * 4 + j + 1]
                if mat == NMAT - 1:
                    nc.scalar.activation(
                        pjg_r_[r][:, j * 256:(j + 1) * 256], ps[:],
                        AF.Sigmoid, bias=bj, scale=1.0)
                else:
                    nc.scalar.activation(
                        pjb_r_[r][:, (mat * 4 + j) * 256:
                                 (mat * 4 + j + 1) * 256], ps[:],
                        AF.Identity, bias=bj, scale=SCALE)

            def stage_gco(g, which):
                """Gate coefficient planes for group g (after its pjg)."""
                r = g % 2
                # on GPSIMD: keeps the ~1us of coefficient building out of
                # the DVE FIFO (it delayed the chain at group boundaries)
                if which == 0:
                    nc.gpsimd.tensor_scalar_mul(gco_r_[r][:, 0:1024],
                                                pjg_r_[r][:], a0)
                    if NREC == 2:
                        nc.gpsimd.tensor_scalar_mul(
                            gco_r_[r][:, 1024:2048], pjg_r_[r][:], a1)
                else:
                    nc.gpsimd.tensor_scalar(g1m_r_[r][:], pjg_r_[r][:],
                                            -1.0, 1.0, ALU.mult, ALU.add)

            # prologue: fully stage group 0
            stage_xdma(0)
            for c in range(4 + NREC * 4):
                stage_proj(0, c)
            stage_gco(0, 0)
            stage_gco(0, 1)
            # init the t=-1 u-tiles: mat slices zero, carry slice = s0
            g1m0r = g1m_r_[0][:].rearrange("p (j t b) -> p j t b", j=4, t=16)
            for h in range(2):
                ui = u_r[h][1]
                nc.vector.memset(ui[:, 0:NREC * 32], 0.0)
                nc.vector.tensor_copy(ui[:, NREC * 32:ME * 32],
                                      s0t[:, h * 32:(h + 1) * 32])
            st_prev = s0t
            for g in range(NG):
                r = g % 2
                pjbr = pjb_r_[r][:].rearrange("p (m j t b) -> p m j t b",
                                              m=NREC, j=4, t=16, b=16)
                gcor = gco_r_[r][:].rearrange("p (m j t b) -> p m j t b",
                                              m=NREC, j=4, t=16, b=16)
                g1mr = g1m_r_[r][:].rearrange("p (j t b) -> p j t b",
                                              j=4, t=16)

                for tt in range(16):
                    t = g * 16 + tt
                    pss = [pss_r[h][t % 3] for h in range(2)]
                    ft = ft_r[t % 2]
                    uu = [u_r[h][t % 2] for h in range(2)]
                    up = [u_r[h][(t + 1) % 2] for h in range(2)]
                    upr = [up[h][:].rearrange("p (m j b) -> p m j b",
                                              m=ME, j=2) for h in range(2)]
                    obuf = ob_r[(t // 4) % 2]
                    st_new = obuf[:, (tt % 4) * 64:(tt % 4) * 64 + 64]

                    # carry slice of this step's u: (1-g) * s_{t-1}
                    for h in range(2):
                        nc.vector.tensor_tensor(
                            uu[h][:, NREC * 32:ME * 32]
                            .rearrange("p (j b) -> p j b", j=2),
                            st_prev[:, h * 32:(h + 1) * 32]
                            .rearrange("p (j b) -> p j b", j=2),
                            g1mr[:, 2 * h:2 * h + 2, tt, :], ALU.mult)

                    # inject bx via fp8-identity matmuls (rhs fp16, exact)
                    for h in range(2):
                        nc.tensor.matmul(
                            pss[h][:].rearrange("p (m j b) -> p m j b",
                                                m=NREC, j=2),
                            iden[:], pjbr[:, :, 2 * h:2 * h + 2, tt, :],
                            start=True, stop=False)

                    # A-matmuls: rhs = 3 m-slices of the previous step's
                    # u-tile (u0, u1, carry); a stride-0 broadcast out AP
                    # makes the PE accumulate them = A @ s_{t-1}
                    def a_mm(h, k, last=False):
                        hk, jk = k // 2, k % 2
                        rhs = upr[hk][:, :, jk, :]
                        for m in range(NREC):
                            for jj in range(2):
                                j = 2 * h + jj
                                blk = (m * 16 + k * 4 + j) * 128
                                out = pss[h][:, m * 32 + jj * 16:
                                             m * 32 + jj * 16 + 16] \
                                    .unsqueeze(1).broadcast_to([128, ME, 16])
                                nc.tensor.matmul(
                                    out, aw[:, blk:blk + 128], rhs,
                                    start=False,
                                    stop=(last and m == NREC - 1
                                          and jj == 1))

                    a_mm(0, 0)
                    a_mm(0, 1)
                    a_mm(0, 2)
                    a_mm(0, 3, last=True)
                    nc.scalar.activation(ft[:, 0:WH], pss[0][:], AF.Tanh,
                                         scale=1.0 / SCALE)
                    a_mm(1, 0)
                    a_mm(1, 1)
                    a_mm(1, 2)
                    a_mm(1, 3, last=True)
                    nc.scalar.activation(ft[:, WH:2 * WH], pss[1][:],
                                         AF.Tanh, scale=1.0 / SCALE)

                    # JIT projections for the next group, spread across
                    # this group's steps (fills PE/ACT idle, keeps PE warm)
                    if g + 1 < NG:
                        if tt == 0:
                            stage_xdma(g + 1)
                        elif 2 <= tt < 2 + 4 + NREC * 4:
                            stage_proj(g + 1, tt - 2)
                        elif tt == 14:
                            stage_gco(g + 1, 0)
                        elif tt == 15:
                            stage_gco(g + 1, 1)

                    # DVE on-path: u_m = ft * gco for both halves
                    for h in range(2):
                        nc.vector.tensor_tensor(
                            uu[h][:, 0:NREC * 32]
                            .rearrange("p (m j b) -> p m j b", m=NREC, j=2),
                            ft[:, h * WH:(h + 1) * WH]
                            .rearrange("p (m j b) -> p m j b", m=NREC, j=2),
                            gcor[:, :, 2 * h:2 * h + 2, tt, :], ALU.mult)
                    # off-path: materialize s_t for output + next carry
                    for h in range(2):
                        with nc.allow_low_precision("fp16 3-term gate sum"):
                            nc.vector.tensor_reduce(
                                st_new[:, h * 32:(h + 1) * 32]
                                .rearrange("p (j b) -> p j b", j=2),
                                uu[h][:].rearrange("p (m j b) -> p j b m",
                                                   m=ME, j=2),
                                mybir.AxisListType.X, ALU.add)

                    st_prev = st_new
                    if tt % 4 == 3:
                        nc.sync.dma_start(
                            stg_d[:, t - 3:t + 1, :],
                            obuf[:].rearrange("p (t c) -> p t c", t=4))

    nc.compile()
    return nc


def _pack_lhsT_blocks(W, kdim, mdim, dtype):
    """W: [mdim*128, kdim*128]; returns [128, kdim*mdim*128] with block
    (k, j) at cols (k*mdim+j)*128 equal to W[j-chunk, k-chunk].T."""
    nk, nj = kdim, mdim
    out = np.zeros((128, nk * nj * 128), dtype=dtype)
    for k in range(nk):
        for j in range(nj):
            blk = W[j * 128:(j + 1) * 128, k * 128:(k + 1) * 128].T
            out[:, (k * nj + j) * 128:(k * nj + j + 1) * 128] = blk
    return np.ascontiguousarray(out)


def kernel(x_seq, s0, A0_w, B0_w, B0_b, A1_w, B1_w, B1_b, gate_w, gate_b,
           alpha, z, _T=None, _trace=False):
    import ml_dtypes
    from concourse.bass_utils import run_bass_kernel_spmd

    T = int(_T or T_FULL)
    alpha_f = float(np.asarray(alpha))
    z_i = int(np.asarray(z))

    key = (alpha_f, z_i, T)
    if key not in _cache:
        _cache[key] = _build(alpha_f, z_i, T)
    nc = _cache[key]

    NMAT = 3 if z_i != 0 else 2
    NREC = 2 if z_i != 0 else 1

    x_seq = np.asarray(x_seq, dtype=np.float32)
    s0 = np.asarray(s0, dtype=np.float32)

    # ---- shared (replicated) weight packing ----
    mats = [np.asarray(B0_w), np.asarray(B1_w), np.asarray(gate_w)][:NMAT] \
        if z_i != 0 else [np.asarray(B0_w), np.asarray(gate_w)]
    biases = [np.asarray(B0_b), np.asarray(B1_b), np.asarray(gate_b)][:NMAT] \
        if z_i != 0 else [np.asarray(B0_b), np.asarray(gate_b)]
    pw = np.concatenate(
        [_pack_lhsT_blocks(W.astype(np.float32), 2, 4, np.float32)
         for W in mats], axis=1).astype(np.float16)
    pw = np.ascontiguousarray(pw)

    # bias for the bx mats is pre-scaled by SCALE (folded into phase-1 ACT)
    bias = np.zeros((128, 4 * NMAT), np.float32)
    for mi, bvec in enumerate(biases):
        scl = 1.0 if mi == NMAT - 1 else SCALE
        bias[:, mi * 4:(mi + 1) * 4] = \
            (scl * bvec.astype(np.float32)).reshape(4, 128).T

    recs = [np.asarray(A0_w)] if z_i == 0 else [np.asarray(A0_w), np.asarray(A1_w)]
    aw = np.concatenate(
        [_pack_lhsT_blocks(A.astype(np.float32), 4, 4, np.float32)
         for A in recs], axis=1) * SCALE
    aw = np.ascontiguousarray(np.clip(aw, -240.0, 240.0)).astype(
        ml_dtypes.float8_e4m3)

    IDEN = np.ascontiguousarray(np.eye(128).astype(ml_dtypes.float8_e4m3))

    # ---- per-core inputs ----
    in_maps = []
    for c in range(N_CORES):
        bc = c * B_LOC
        xc = x_seq[bc:bc + B_LOC, :T]                       # [16, T, 256]
        xT = np.ascontiguousarray(
            xc.transpose(2, 1, 0).reshape(2, 128, T * B_LOC)).astype(
                np.float16)
        s0c = s0[bc:bc + B_LOC]                             # [16, 512]
        s0T = np.ascontiguousarray(
            s0c.T.reshape(4, 128, B_LOC).transpose(1, 0, 2).reshape(128, 64)
        ).astype(np.float16)
        in_maps.append({
            "xT": xT, "pw": pw, "bias": bias, "aw": aw, "s0T": s0T,
            "iden": IDEN,
        })

    res = run_bass_kernel_spmd(nc, in_maps, list(range(N_CORES)), trace=_trace)
    if _trace:
        kernel._last_res = res

    out = np.empty((B_FULL, T + 1, S_DIM), np.float32)
    for c in range(N_CORES):
        bc = c * B_LOC
        stg = np.asarray(res.results[c]["stg"]).astype(np.float32)
        out[bc:bc + B_LOC, 0] = s0[bc:bc + B_LOC]
        out[bc:bc + B_LOC, 1:] = (
            stg.reshape(128, T, 4, B_LOC).transpose(3, 1, 2, 0)
            .reshape(B_LOC, T, S_DIM))
    return out
